# revision 8
# baseline (speedup 1.0000x reference)
"""MixHop GNN (3 layers + final linear) on 8 Trainium2 NeuronCores.

Math restructuring (validated vs reference to ~7e-7 rel in f64):
  - gcn_norm factored: norm_e = dinv[src]*dinv[dst]; propagation becomes
      y = dinv * (sum_{e: dst=i} zp[src_e] + zp[i]),  zp = dinv * z
    over the *binary* adjacency (self loop handled densely).
  - (A h) @ W == A (h @ W): propagate 60/120-col z-tables, not 128/180-col h.
  - Per layer: out0 = h@W0, out1 = A z1, out2 = A (A z2); hop1 propagates
    [z1|z2] (120 cols, 512B rows), hop2 propagates q = A z2 (60->64 cols).
  - BatchNorm folded into the next layer's matmuls: BN(h)@W = h@(s*W) + tv@W
    (s = g*rsqrt(var+eps), tv = beta - mu*s); the tv@W row rides as an extra
    contraction row (ones row in hT). Conv biases vanish under BN shift
    invariance; lin_b is added on the host.

Distribution: nodes sharded by range across 8 cores (dst-sharded edges);
row-wise dense compute shard-local; per-hop tables exchanged via AllGather;
BN stats via a small AllReduce.

Gathers: nc.gpsimd.dma_gather with host-precomputed int16 index lists.
Nodes are permuted by in-degree so per-(tile,partition) edge-slot padding is
small and uniform across cores (SPMD-uniform program). int16 index range is
handled by splitting tables into lo/hi halves with rebased indices; each
half has a guaranteed all-zero row as pad target. Segment sum = strided DVE
tensor_reduce over per-node slots.
"""

import numpy as np

C = 8
P = 128
N_REAL = 50000
FIN = 128
HID = 60
CAT = 3 * HID
FB = 64          # 64-aligned feature block stride in hT / packed weights
PCAT = 3 * FB    # 192
KB = PCAT - 128  # 64 rows in the second hT chunk (+1 bias row on device)
ZC = 2 * HID
TW = 128
QW = 64
EPS = 1e-5

TPC = 49
NPC = TPC * P
NP = C * NPC
HALF = NP // 2
SL = 32


# ============================================================ host preprocess
def preprocess(x, edge_index, n_real=N_REAL, tpc=TPC, sl=SL):
    np_total = C * tpc * P
    npc = tpc * P
    half = np_total // 2
    src = np.asarray(edge_index[0], dtype=np.int64)
    dst = np.asarray(edge_index[1], dtype=np.int64)

    deg = np.bincount(dst, minlength=n_real).astype(np.float64) + 1.0
    dinv = (1.0 / np.sqrt(deg)).astype(np.float32)

    order = np.argsort(-deg, kind="stable")
    slots = np.full(np_total, -1, dtype=np.int64)
    nblocks = (n_real + P - 1) // P
    assert nblocks <= C * tpc
    for k in range(nblocks):
        nodes = order[k * P : (k + 1) * P]
        c, t = k % C, k // C
        base = c * npc + t * P
        slots[base : base + len(nodes)] = nodes
    if slots[0] >= 0:
        j = np_total - 2
        while slots[j] >= 0:
            j -= 1
        slots[0], slots[j] = slots[j], slots[0]
    assert slots[np_total - 1] == -1, "need an all-zero row in the hi half"
    inv = np.full(n_real, -1, dtype=np.int64)
    realmask = slots >= 0
    inv[slots[realmask]] = np.nonzero(realmask)[0]

    s2 = inv[src]
    d2 = inv[dst]

    dinv_new = np.zeros(np_total, dtype=np.float32)
    dinv_new[realmask] = dinv[slots[realmask]]
    mask_new = realmask.astype(np.float32)

    ecore = d2 // npc
    et = (d2 % npc) // P
    ep = d2 % P
    is_lo = s2 < half

    cnt = np.zeros((2, C, tpc, P), dtype=np.int64)
    np.add.at(cnt, (1 - is_lo.astype(np.int64), ecore, et, ep), 1)
    L_lo = cnt[0].max(axis=(0, 2)).astype(int)
    L_hi = cnt[1].max(axis=(0, 2)).astype(int)

    chunks = []
    cur, clo, chi = [], 0, 0
    for t in range(tpc):
        if cur and (clo + L_lo[t] > sl or chi + L_hi[t] > sl):
            chunks.append(dict(tiles=cur, slo=clo, shi=chi))
            cur, clo, chi = [], 0, 0
        cur.append(t)
        clo += int(L_lo[t])
        chi += int(L_hi[t])
    if cur:
        chunks.append(dict(tiles=cur, slo=clo, shi=chi))

    tot = 0
    for cm in chunks:
        cm["col"] = tot
        b = 0
        cm["lo_base"] = {}
        for t in cm["tiles"]:
            cm["lo_base"][t] = b
            b += int(L_lo[t])
        b = 0
        cm["hi_base"] = {}
        for t in cm["tiles"]:
            cm["hi_base"][t] = b
            b += int(L_hi[t])
        tot += (cm["slo"] + cm["shi"]) * P // 16
    tot_slots = tot

    # flat per-core index stream, then 16-wrap
    pad_lo, pad_hi = 0, np_total - 1 - half
    flat = np.zeros((C, tot_slots * 16), dtype=np.int64)
    for cm in chunks:
        o = cm["col"] * 16
        n_lo = cm["slo"] * P
        n_hi = cm["shi"] * P
        flat[:, o : o + n_lo] = pad_lo
        flat[:, o + n_lo : o + n_lo + n_hi] = pad_hi

    key = ((ecore * tpc + et) * P + ep) * 2 + (1 - is_lo.astype(np.int64))
    eorder = np.argsort(key, kind="stable")
    key_s = key[eorder]
    first = np.ones(len(key_s), dtype=bool)
    first[1:] = key_s[1:] != key_s[:-1]
    start_pos = np.nonzero(first)[0]
    grp = np.cumsum(first) - 1
    rank = np.arange(len(key_s)) - start_pos[grp]

    tile2chunk = np.zeros(tpc, dtype=int)
    for ci, cm in enumerate(chunks):
        for t in cm["tiles"]:
            tile2chunk[t] = ci
    chunk_col = np.array([chunks[tile2chunk[t]]["col"] for t in range(tpc)])
    chunk_slo = np.array([chunks[tile2chunk[t]]["slo"] for t in range(tpc)])
    lo_base = np.array([chunks[tile2chunk[t]]["lo_base"][t] for t in range(tpc)])
    hi_base = np.array([chunks[tile2chunk[t]]["hi_base"][t] for t in range(tpc)])

    ec_s = ecore[eorder]
    et_s = et[eorder]
    ep_s = ep[eorder]
    lo_s = is_lo[eorder]
    s2_s = s2[eorder]
    base_in_chunk = np.where(lo_s, lo_base[et_s], chunk_slo[et_s] + hi_base[et_s])
    pos = chunk_col[et_s] * 16 + (base_in_chunk + rank) * P + ep_s
    val = np.where(lo_s, s2_s, s2_s - half)
    flat[ec_s, pos] = val
    assert flat.min() >= 0 and flat.max() < half

    wrapped = flat.reshape(C, tot_slots, 16).transpose(0, 2, 1)
    idx_arr = np.ascontiguousarray(
        np.tile(wrapped, (1, 8, 1)).astype(np.int16)
    )

    x = np.asarray(x, dtype=np.float32)
    x_perm = np.zeros((np_total, x.shape[1]), dtype=np.float32)
    x_perm[realmask] = x[slots[realmask]]
    xT = np.ascontiguousarray(x_perm.T)

    dinv_t = np.ascontiguousarray(
        dinv_new.reshape(C, tpc, P).transpose(0, 2, 1)
    )
    mask_t = np.ascontiguousarray(
        mask_new.reshape(C, tpc, P).transpose(0, 2, 1)
    )

    meta = dict(
        L_lo=L_lo,
        L_hi=L_hi,
        chunks=chunks,
        tot_slots=tot_slots,
        slots=slots,
        realmask=realmask,
        n_real=n_real,
        npc=npc,
        tpc=tpc,
        np_total=np_total,
        half=half,
    )
    arrays = dict(
        idx=idx_arr, xT=xT, dinv=dinv_t, dinv2=dinv_t * dinv_t, mask=mask_t
    )
    return meta, arrays


# ================================================================= builder
def build_nc(meta):
    from contextlib import ExitStack

    import concourse.mybir as mybir
    import concourse.tile as tile
    from concourse import bacc
    from concourse.masks import make_identity

    f32 = mybir.dt.float32
    i16 = mybir.dt.int16
    OP = mybir.AluOpType
    AX = mybir.AxisListType
    AF = mybir.ActivationFunctionType

    tpc = meta["tpc"]
    npc = meta["npc"]
    np_total = meta["np_total"]
    half = meta["half"]
    tot_slots = meta["tot_slots"]
    L_lo, L_hi = meta["L_lo"], meta["L_hi"]
    chunks = meta["chunks"]
    inv_n = 1.0 / meta["n_real"]

    nc = bacc.Bacc(None, target_bir_lowering=False)

    xT_d = nc.dram_tensor("xT", [FIN, npc], f32, kind="ExternalInput")
    idx_d = nc.dram_tensor("idx", [P, tot_slots], i16, kind="ExternalInput")
    dinv_d = nc.dram_tensor("dinv", [P, tpc], f32, kind="ExternalInput")
    dinv2_d = nc.dram_tensor("dinv2", [P, tpc], f32, kind="ExternalInput")
    mask_d = nc.dram_tensor("mask", [P, tpc], f32, kind="ExternalInput")
    w1z_d = nc.dram_tensor("w1z", [FIN, ZC], f32, kind="ExternalInput")
    w10_d = nc.dram_tensor("w10", [FIN, HID], f32, kind="ExternalInput")
    wz_a_d, wz_b_d, w0_a_d, w0_b_d = {}, {}, {}, {}
    for L in (2, 3):
        wz_a_d[L] = nc.dram_tensor(f"w{L}z_a", [P, ZC], f32, kind="ExternalInput")
        wz_b_d[L] = nc.dram_tensor(f"w{L}z_b", [KB, ZC], f32, kind="ExternalInput")
        w0_a_d[L] = nc.dram_tensor(f"w{L}0_a", [P, HID], f32, kind="ExternalInput")
        w0_b_d[L] = nc.dram_tensor(f"w{L}0_b", [KB, HID], f32, kind="ExternalInput")
    lin_a_d = nc.dram_tensor("lin_a", [P, 8], f32, kind="ExternalInput")
    lin_b_d = nc.dram_tensor("lin_b2", [KB, 8], f32, kind="ExternalInput")
    bng_d, bnb_d = {}, {}
    for L in (1, 2, 3):
        bng_d[L] = nc.dram_tensor(f"bn{L}g", [1, PCAT], f32, kind="ExternalInput")
        bnb_d[L] = nc.dram_tensor(f"bn{L}b", [1, PCAT], f32, kind="ExternalInput")
    out_d = nc.dram_tensor("logits", [npc, 8], f32, kind="ExternalOutput")

    zzsh = nc.dram_tensor("zzsh", [npc, TW], f32)
    T = nc.dram_tensor("Ttab", [np_total, TW], f32, addr_space="Shared")
    qsh = nc.dram_tensor("qsh", [npc, QW], f32)
    Q = nc.dram_tensor("Qtab", [np_total, QW], f32, addr_space="Shared")
    stats_in = nc.dram_tensor("stats_in", [2, PCAT], f32)
    stats_out = nc.dram_tensor("stats_out", [2, PCAT], f32, addr_space="Shared")

    rg = [list(range(C))]

    with tile.TileContext(nc) as tc, ExitStack() as ex:
        cpool = ex.enter_context(tc.tile_pool(name="const", bufs=1))
        zpool = ex.enter_context(tc.tile_pool(name="zwork", bufs=3))
        ppool = ex.enter_context(tc.tile_pool(name="psum", bufs=2, space="PSUM"))
        spsum = ex.enter_context(tc.tile_pool(name="spsum", bufs=1, space="PSUM"))
        gpool = ex.enter_context(tc.tile_pool(name="gather", bufs=2))
        hpool = ex.enter_context(tc.tile_pool(name="hopwork", bufs=4))
        lpool = ex.enter_context(tc.tile_pool(name="layer", bufs=2))

        def P_(shape, name, tag):
            return ppool.tile(shape, f32, name=name, tag=tag)

        idx_t = cpool.tile([P, tot_slots], i16, name="idx_t")
        nc.sync.dma_start(out=idx_t[:], in_=idx_d[:, :])
        dinv_t = cpool.tile([P, tpc], f32, name="dinv_t")
        nc.sync.dma_start(out=dinv_t[:], in_=dinv_d[:, :])
        dinv2_t = cpool.tile([P, tpc], f32, name="dinv2_t")
        nc.sync.dma_start(out=dinv2_t[:], in_=dinv2_d[:, :])
        mask_t = cpool.tile([P, tpc], f32, name="mask_t")
        nc.sync.dma_start(out=mask_t[:], in_=mask_d[:, :])
        ident = cpool.tile([P, P], f32, name="ident")
        make_identity(nc, ident[:])
        one11 = cpool.tile([1, 1], f32, name="one11")
        nc.vector.memset(one11[:], 1.0)

        out0_sb = cpool.tile([P, tpc * HID], f32, name="out0_sb")
        out1_sb = cpool.tile([P, tpc * HID], f32, name="out1_sb")
        out2_sb = cpool.tile([P, tpc * HID], f32, name="out2_sb")
        zself = cpool.tile([P, tpc * TW], f32, name="zself")
        qself = cpool.tile([P, tpc * QW], f32, name="qself")
        nc.vector.memset(zself[:], 0.0)
        nc.vector.memset(qself[:], 0.0)
        out_sbs = (out0_sb, out1_sb, out2_sb)

        wz_a_t, wz_b_t, w0_a_t, w0_b_t = {}, {}, {}, {}
        for L in (2, 3):
            wz_a_t[L] = cpool.tile([P, ZC], f32, name=f"wza{L}")
            nc.sync.dma_start(out=wz_a_t[L][:], in_=wz_a_d[L][:, :])
            wz_b_t[L] = cpool.tile([KB, ZC], f32, name=f"wzb{L}")
            nc.sync.dma_start(out=wz_b_t[L][:], in_=wz_b_d[L][:, :])
            w0_a_t[L] = cpool.tile([P, HID], f32, name=f"w0a{L}")
            nc.sync.dma_start(out=w0_a_t[L][:], in_=w0_a_d[L][:, :])
            w0_b_t[L] = cpool.tile([KB, HID], f32, name=f"w0b{L}")
            nc.sync.dma_start(out=w0_b_t[L][:], in_=w0_b_d[L][:, :])
        lin_a_t = cpool.tile([P, 8], f32, name="lina")
        nc.sync.dma_start(out=lin_a_t[:], in_=lin_a_d[:, :])
        lin_b_t = cpool.tile([KB, 8], f32, name="linb")
        nc.sync.dma_start(out=lin_b_t[:], in_=lin_b_d[:, :])
        w1z_t = cpool.tile([FIN, ZC], f32, name="w1z")
        nc.sync.dma_start(out=w1z_t[:], in_=w1z_d[:, :])
        w10_t = cpool.tile([FIN, HID], f32, name="w10")
        nc.sync.dma_start(out=w10_t[:], in_=w10_d[:, :])
        bng_t, bnb_t = {}, {}
        for L in (1, 2, 3):
            bng_t[L] = cpool.tile([1, PCAT], f32, name=f"bng{L}")
            nc.sync.dma_start(out=bng_t[L][:], in_=bng_d[L][:, :])
            bnb_t[L] = cpool.tile([1, PCAT], f32, name=f"bnb{L}")
            nc.sync.dma_start(out=bnb_t[L][:], in_=bnb_d[L][:, :])

        # ---------------------------------------------------------- helpers
        def transpose_h(t, tag):
            """hT chunks [128,128] (out0T@0, out1T@64) + [65,128]
            (out2T@0, ones row @64); pad rows zeroed (weights there are 0,
            but NaN*0 must be avoided)."""
            hta = zpool.tile([P, P], f32, name=f"hta_{tag}", tag="hta")
            htb = zpool.tile([KB + 1, P], f32, name=f"htb_{tag}", tag="htb")
            nc.vector.memset(hta[:], 0.0)
            nc.vector.memset(htb[:], 0.0)
            for i, src_sb in enumerate(out_sbs):
                ps = P_([HID, P], f"tp_{tag}_{i}", "tp")
                nc.tensor.transpose(
                    out=ps[:],
                    in_=src_sb[:, t * HID : (t + 1) * HID],
                    identity=ident[:],
                )
                if i < 2:
                    nc.vector.tensor_copy(out=hta[i * FB : i * FB + HID, :], in_=ps[:])
                else:
                    nc.vector.tensor_copy(out=htb[0:HID, :], in_=ps[:])
            nc.vector.memset(htb[KB : KB + 1, :], 1.0)
            return hta, htb

        def z_phase(L, sWz_a, sWz_b, sW0_a, sW0_b):
            for t in range(tpc):
                if L == 1:
                    hta = zpool.tile([FIN, P], f32, name=f"hx_{t}", tag="hta")
                    nc.sync.dma_start(out=hta[:], in_=xT_d[:, t * P : (t + 1) * P])
                    htb = None
                else:
                    hta, htb = transpose_h(t, f"z{L}_{t}")
                zp = P_([P, ZC], f"zp_{L}_{t}", "zp")
                nc.tensor.matmul(
                    out=zp[:], lhsT=hta[:], rhs=sWz_a[:], start=True, stop=htb is None
                )
                if htb is not None:
                    nc.tensor.matmul(
                        out=zp[:], lhsT=htb[:], rhs=sWz_b[:], start=False, stop=True
                    )
                op = P_([P, HID], f"op_{L}_{t}", "op")
                nc.tensor.matmul(
                    out=op[:], lhsT=hta[:], rhs=sW0_a[:], start=True, stop=htb is None
                )
                if htb is not None:
                    nc.tensor.matmul(
                        out=op[:], lhsT=htb[:], rhs=sW0_b[:], start=False, stop=True
                    )
                nc.vector.tensor_scalar(
                    out=zself[:, t * TW : t * TW + ZC],
                    in0=zp[:],
                    scalar1=dinv_t[:, t : t + 1],
                    scalar2=None,
                    op0=OP.mult,
                )
                nc.vector.tensor_copy(
                    out=out0_sb[:, t * HID : (t + 1) * HID], in_=op[:]
                )
                nc.sync.dma_start(
                    out=zzsh[t * P : (t + 1) * P, :],
                    in_=zself[:, t * TW : (t + 1) * TW],
                )

        def hop(tab, elem, key, self_sb, self_w, epilogue):
            for ci, cm in enumerate(chunks):
                slo, shi = cm["slo"], cm["shi"]
                nlo, nhi = slo * P, shi * P
                gb = gpool.tile(
                    [P, (slo + shi) * elem], f32, name=f"gb_{key}_{ci}", tag="gb"
                )
                co = cm["col"]
                if nlo:
                    nc.gpsimd.dma_gather(
                        out_ap=gb[:, : slo * elem].rearrange(
                            "p (j e) -> p j e", e=elem
                        ),
                        in_ap=tab[0:half, :],
                        idxs_ap=idx_t[:, co : co + nlo // 16],
                        num_idxs=nlo,
                        num_idxs_reg=nlo,
                        elem_size=elem,
                        single_packet=False,
                    )
                if nhi:
                    nc.gpsimd.dma_gather(
                        out_ap=gb[:, slo * elem :].rearrange(
                            "p (j e) -> p j e", e=elem
                        ),
                        in_ap=tab[half:np_total, :],
                        idxs_ap=idx_t[:, co + nlo // 16 : co + (nlo + nhi) // 16],
                        num_idxs=nhi,
                        num_idxs_reg=nhi,
                        elem_size=elem,
                        single_packet=False,
                    )
                for t in cm["tiles"]:
                    terms = []
                    if L_lo[t]:
                        ylo = hpool.tile([P, elem], f32, name=f"ylo_{key}_{t}", tag="ylo")
                        b = cm["lo_base"][t]
                        nc.vector.tensor_reduce(
                            out=ylo[:],
                            in_=gb[:, b * elem : (b + L_lo[t]) * elem].rearrange(
                                "p (j e) -> p e j", e=elem
                            ),
                            axis=AX.X,
                            op=OP.add,
                        )
                        terms.append(ylo)
                    if L_hi[t]:
                        yhi = hpool.tile([P, elem], f32, name=f"yhi_{key}_{t}", tag="yhi")
                        b = slo + cm["hi_base"][t]
                        nc.vector.tensor_reduce(
                            out=yhi[:],
                            in_=gb[:, b * elem : (b + L_hi[t]) * elem].rearrange(
                                "p (j e) -> p e j", e=elem
                            ),
                            axis=AX.X,
                            op=OP.add,
                        )
                        terms.append(yhi)
                    s = hpool.tile([P, elem], f32, name=f"s_{key}_{t}", tag="s")
                    own = self_sb[:, t * self_w : t * self_w + elem]
                    if len(terms) == 2:
                        nc.vector.tensor_tensor(
                            out=s[:], in0=terms[0][:], in1=terms[1][:], op=OP.add
                        )
                        nc.vector.tensor_tensor(out=s[:], in0=s[:], in1=own, op=OP.add)
                    elif len(terms) == 1:
                        nc.vector.tensor_tensor(
                            out=s[:], in0=terms[0][:], in1=own, op=OP.add
                        )
                    else:
                        nc.vector.tensor_copy(out=s[:], in_=own)
                    epilogue(t, s)

        def stats_phase(L):
            ssum = spsum.tile([1, PCAT], f32, name=f"ssum{L}", tag="ssum")
            ssq = spsum.tile([1, PCAT], f32, name=f"ssq{L}", tag="ssq")
            for t in range(tpc):
                for i, sb in enumerate(out_sbs):
                    xt = sb[:, t * HID : (t + 1) * HID]
                    nc.tensor.matmul(
                        out=ssum[:, i * FB : i * FB + HID],
                        lhsT=mask_t[:, t : t + 1],
                        rhs=xt,
                        start=(t == 0 and i == 0),
                        stop=(t == tpc - 1 and i == 2),
                    )
                    sq = hpool.tile([P, HID], f32, name=f"sq_{L}_{t}_{i}", tag="sq")
                    nc.vector.tensor_tensor(out=sq[:], in0=xt, in1=xt, op=OP.mult)
                    nc.tensor.matmul(
                        out=ssq[:, i * FB : i * FB + HID],
                        lhsT=mask_t[:, t : t + 1],
                        rhs=sq[:],
                        start=(t == 0 and i == 0),
                        stop=(t == tpc - 1 and i == 2),
                    )
            sio_s = lpool.tile([1, PCAT], f32, name=f"sios{L}", tag="sios")
            sio_q = lpool.tile([1, PCAT], f32, name=f"sioq{L}", tag="sioq")
            nc.vector.memset(sio_s[:], 0.0)
            nc.vector.memset(sio_q[:], 0.0)
            for i in range(3):
                nc.vector.tensor_copy(
                    out=sio_s[:, i * FB : i * FB + HID],
                    in_=ssum[:, i * FB : i * FB + HID],
                )
                nc.vector.tensor_copy(
                    out=sio_q[:, i * FB : i * FB + HID],
                    in_=ssq[:, i * FB : i * FB + HID],
                )
            nc.sync.dma_start(out=stats_in[0:1, :], in_=sio_s[:])
            nc.sync.dma_start(out=stats_in[1:2, :], in_=sio_q[:])
            nc.gpsimd.collective_compute(
                "AllReduce",
                OP.add,
                ins=[stats_in.ap().opt()],
                outs=[stats_out.ap().opt()],
                replica_groups=rg,
            )
            sor_s = lpool.tile([1, PCAT], f32, name=f"sors{L}", tag="sors")
            sor_q = lpool.tile([1, PCAT], f32, name=f"sorq{L}", tag="sorq")
            nc.sync.dma_start(out=sor_s[:], in_=stats_out[0:1, :])
            nc.sync.dma_start(out=sor_q[:], in_=stats_out[1:2, :])
            return sor_s, sor_q

        def bn_fold(L, sor_pair, wspecs):
            """wspecs: list of (wa, wb, width, with_bias). Returns scaled
            chunk pairs (sWa [P,w], sWb [KB+1,w] incl bias row)."""
            sor_s, sor_q = sor_pair
            mu = lpool.tile([1, PCAT], f32, name=f"mu{L}", tag="mu")
            nc.vector.tensor_scalar(
                out=mu[:], in0=sor_s[:], scalar1=inv_n, scalar2=None, op0=OP.mult
            )
            var = lpool.tile([1, PCAT], f32, name=f"var{L}", tag="var")
            nc.vector.tensor_scalar(
                out=var[:], in0=sor_q[:], scalar1=inv_n, scalar2=None, op0=OP.mult
            )
            mu2 = lpool.tile([1, PCAT], f32, name=f"mu2{L}", tag="mu2")
            nc.vector.tensor_tensor(out=mu2[:], in0=mu[:], in1=mu[:], op=OP.mult)
            nc.vector.tensor_tensor(out=var[:], in0=var[:], in1=mu2[:], op=OP.subtract)
            nc.vector.tensor_scalar(
                out=var[:], in0=var[:], scalar1=EPS, scalar2=None, op0=OP.add
            )
            sq = lpool.tile([1, PCAT], f32, name=f"sqv{L}", tag="sqv")
            nc.scalar.activation(out=sq[:], in_=var[:], func=AF.Sqrt)
            rs = lpool.tile([1, PCAT], f32, name=f"rs{L}", tag="rs")
            nc.vector.reciprocal(out=rs[:], in_=sq[:])
            sv = lpool.tile([1, PCAT], f32, name=f"sv{L}", tag="sv")
            nc.vector.tensor_tensor(out=sv[:], in0=rs[:], in1=bng_t[L][:], op=OP.mult)
            tv = lpool.tile([1, PCAT], f32, name=f"tv{L}", tag="tv")
            nc.vector.tensor_tensor(out=tv[:], in0=mu[:], in1=sv[:], op=OP.mult)
            nc.vector.tensor_tensor(
                out=tv[:], in0=bnb_t[L][:], in1=tv[:], op=OP.subtract
            )
            sT_a = lpool.tile([P, 1], f32, name=f"sTa{L}", tag="sTa")
            sT_b = lpool.tile([KB, 1], f32, name=f"sTb{L}", tag="sTb")
            tT_a = lpool.tile([P, 1], f32, name=f"tTa{L}", tag="tTa")
            tT_b = lpool.tile([KB, 1], f32, name=f"tTb{L}", tag="tTb")
            for dst_t, row, c0, c1 in (
                (sT_a, sv, 0, P),
                (tT_a, tv, 0, P),
                (sT_b, sv, P, PCAT),
                (tT_b, tv, P, PCAT),
            ):
                psx = P_([c1 - c0, 1], f"psx_{L}_{c0}_{id(dst_t):x}", "tp")
                nc.tensor.matmul(
                    out=psx[:], lhsT=row[0:1, c0:c1], rhs=one11[:], start=True, stop=True
                )
                nc.vector.tensor_copy(out=dst_t[:], in_=psx[:])

            outs = []
            for wa, wb, width, with_bias in wspecs:
                sWa = lpool.tile(
                    [P, width], f32, name=f"sWa{L}_{width}_{int(with_bias)}",
                    tag=f"sWa{width}{int(with_bias)}",
                )
                nc.vector.tensor_scalar(
                    out=sWa[:], in0=wa[:], scalar1=sT_a[:], scalar2=None, op0=OP.mult
                )
                sWb = lpool.tile(
                    [KB + 1, width], f32, name=f"sWb{L}_{width}_{int(with_bias)}",
                    tag=f"sWb{width}{int(with_bias)}",
                )
                nc.vector.tensor_scalar(
                    out=sWb[:KB, :], in0=wb[:], scalar1=sT_b[:], scalar2=None,
                    op0=OP.mult,
                )
                if with_bias:
                    bp = P_([1, width], f"bp{L}_{width}", "tp")
                    nc.tensor.matmul(
                        out=bp[:], lhsT=tT_a[:], rhs=wa[:], start=True, stop=False
                    )
                    nc.tensor.matmul(
                        out=bp[:], lhsT=tT_b[:], rhs=wb[:], start=False, stop=True
                    )
                    nc.vector.tensor_copy(out=sWb[KB : KB + 1, :], in_=bp[:])
                else:
                    nc.vector.memset(sWb[KB : KB + 1, :], 0.0)
                outs.append((sWa, sWb))
            return outs

        # ------------------------------------------------------ main pipeline
        cur = dict(wz=(w1z_t, None), w0=(w10_t, None))
        fin = {}
        for L in (1, 2, 3):
            z_phase(L, cur["wz"][0], cur["wz"][1], cur["w0"][0], cur["w0"][1])
            nc.gpsimd.collective_compute(
                "AllGather",
                OP.bypass,
                ins=[zzsh.ap().opt()],
                outs=[T.ap().opt()],
                replica_groups=rg,
            )

            def hop1_epi(t, s):
                nc.vector.tensor_scalar(
                    out=out1_sb[:, t * HID : (t + 1) * HID],
                    in0=s[:, 0:HID],
                    scalar1=dinv_t[:, t : t + 1],
                    scalar2=None,
                    op0=OP.mult,
                )
                nc.vector.tensor_scalar(
                    out=qself[:, t * QW : t * QW + HID],
                    in0=s[:, HID:ZC],
                    scalar1=dinv2_t[:, t : t + 1],
                    scalar2=None,
                    op0=OP.mult,
                )
                nc.sync.dma_start(
                    out=qsh[t * P : (t + 1) * P, :],
                    in_=qself[:, t * QW : (t + 1) * QW],
                )

            hop(T, TW, f"a{L}", zself, TW, hop1_epi)
            nc.gpsimd.collective_compute(
                "AllGather",
                OP.bypass,
                ins=[qsh.ap().opt()],
                outs=[Q.ap().opt()],
                replica_groups=rg,
            )

            def hop2_epi(t, s):
                nc.vector.tensor_scalar(
                    out=out2_sb[:, t * HID : (t + 1) * HID],
                    in0=s[:, 0:HID],
                    scalar1=dinv_t[:, t : t + 1],
                    scalar2=None,
                    op0=OP.mult,
                )

            hop(Q, QW, f"b{L}", qself, QW, hop2_epi)

            sor = stats_phase(L)
            if L < 3:
                nL = L + 1
                res = bn_fold(
                    L,
                    sor,
                    [
                        (wz_a_t[nL], wz_b_t[nL], ZC, True),
                        (w0_a_t[nL], w0_b_t[nL], HID, False),
                    ],
                )
                cur = dict(wz=res[0], w0=res[1])
            else:
                fin["w"] = bn_fold(L, sor, [(lin_a_t, lin_b_t, 8, True)])[0]

        fin_a, fin_b = fin["w"]
        for t in range(tpc):
            hta, htb = transpose_h(t, f"fin_{t}")
            lp = P_([P, 8], f"lp_{t}", "op")
            nc.tensor.matmul(out=lp[:], lhsT=hta[:], rhs=fin_a[:], start=True, stop=False)
            nc.tensor.matmul(out=lp[:], lhsT=htb[:], rhs=fin_b[:], start=False, stop=True)
            lo = zpool.tile([P, 8], f32, name=f"lot_{t}", tag="lout")
            nc.vector.tensor_copy(out=lo[:], in_=lp[:])
            nc.sync.dma_start(out=out_d[t * P : (t + 1) * P, :], in_=lo[:])

    nc.finalize()
    return nc


# ================================================================ weights
def _pad_rows(w):
    """[CAT, width] -> [PCAT, width]: block i of 60 rows lands at 64*i."""
    out = np.zeros((PCAT, w.shape[1]), np.float32)
    for i in range(3):
        out[i * FB : i * FB + HID] = w[i * HID : (i + 1) * HID]
    return out


def _pad_cols(v):
    out = np.zeros((1, PCAT), np.float32)
    for i in range(3):
        out[0, i * FB : i * FB + HID] = v[i * HID : (i + 1) * HID]
    return out


def pack_weights(inputs):
    f = np.float32
    w = {}
    w["w1z"] = np.ascontiguousarray(
        np.concatenate([inputs["c1_w1"], inputs["c1_w2"]], axis=1), dtype=f
    )
    w["w10"] = np.ascontiguousarray(inputs["c1_w0"], dtype=f)
    for L in (2, 3):
        wz = _pad_rows(
            np.concatenate([inputs[f"c{L}_w1"], inputs[f"c{L}_w2"]], axis=1).astype(f)
        )
        w[f"w{L}z_a"] = np.ascontiguousarray(wz[:P])
        w[f"w{L}z_b"] = np.ascontiguousarray(wz[P:])
        w0 = _pad_rows(np.asarray(inputs[f"c{L}_w0"], f))
        w[f"w{L}0_a"] = np.ascontiguousarray(w0[:P])
        w[f"w{L}0_b"] = np.ascontiguousarray(w0[P:])
    lw = np.zeros((CAT, 8), f)
    lw[:, :7] = np.asarray(inputs["lin_w"], f)
    lw = _pad_rows(lw)
    w["lin_a"] = np.ascontiguousarray(lw[:P])
    w["lin_b2"] = np.ascontiguousarray(lw[P:])
    for L in (1, 2, 3):
        w[f"bn{L}g"] = _pad_cols(np.asarray(inputs[f"bn{L}_g"], f))
        w[f"bn{L}b"] = _pad_cols(np.asarray(inputs[f"bn{L}_b"], f))
    return w


def make_in_maps(meta, arrays, w):
    npc = meta["npc"]
    in_maps = []
    for c in range(C):
        m = dict(w)
        m["xT"] = np.ascontiguousarray(arrays["xT"][:, c * npc : (c + 1) * npc])
        m["idx"] = arrays["idx"][c]
        m["dinv"] = arrays["dinv"][c]
        m["dinv2"] = arrays["dinv2"][c]
        m["mask"] = arrays["mask"][c]
        in_maps.append(m)
    return in_maps


def assemble_output(meta, results, lin_b):
    logits_p = np.concatenate([r["logits"] for r in results], axis=0)
    out = np.zeros((meta["n_real"], 7), dtype=np.float32)
    rm = meta["realmask"]
    out[meta["slots"][rm]] = logits_p[rm][:, :7]
    out += np.asarray(lin_b, dtype=np.float32)
    return out


# ================================================================== kernel
def kernel(**inputs):
    from concourse.bass_utils import run_bass_kernel_spmd

    x = np.asarray(inputs["x"], dtype=np.float32)
    edge_index = np.asarray(inputs["edge_index"])
    meta, arrays = preprocess(x, edge_index)
    nc = build_nc(meta)
    w = pack_weights(inputs)
    in_maps = make_in_maps(meta, arrays, w)
    res = run_bass_kernel_spmd(nc, in_maps, core_ids=list(range(C)))
    return assemble_output(meta, res.results, inputs["lin_b"])
